# revision 55
# baseline (speedup 1.0000x reference)
"""Multi-head attention (B=2, N=4096, C=512, H=8, d=64) on 8 Trainium2 NeuronCores.

Sharding: core c handles batch b = c//4 and heads {2*(c%4), 2*(c%4)+1}.
Each core computes its 2 heads' attention plus a partial output projection
(contraction over its 128 rows of W_proj); the host gather sums the 4
partials per batch and adds the bias.

On-device dataflow (transposed-scores formulation, no on-chip transposes):
  qT/kT [128=2*64 d-dims, 4096]  = W.T @ x.T      (x.T supplied by host)
  v_aug [128 n-chunk, 32*(65+65)] = x @ Wv with a ones column per head
  S^T[kidx, q] = kT.T_chunk @ qT  (two heads ride row-groups 0-1 / 2-3
                                   of the PE array concurrently, K=64 each)
  E = exp(S^T / 8)                softmax exp split across TWO engines:
                                  most steps on ScalarE (LUT exp, scale
                                  folded), ~7/32 steps per block on the
                                  Vector engine via two custom DVE ops:
                                  q = cubic(s) (minimax for e^{s/256})
                                  then est = (q + d0)^32 (5 squarings).
                                  ScalarE is the critical engine
                                  (1 elem/lane/cycle); offloading part
                                  of the exp volume to DVE breaks that
                                  ceiling.
  [out_unnorm^T; den] = v_aug.T @ E   (ones column makes row 64 the softmax
                                       denominator -- no extra pass)
  out^T = out_unnorm^T * (1/den)  (fast approx reciprocal + K=1 broadcast
                                   matmul)
  partial = out^T.T @ W_proj_slice  (per-head K=64 contractions; bias is
                                     added by the host gather)
"""

import sys
import types

for _p in ("/opt/trn_rl_repo",):
    if _p not in sys.path:
        sys.path.insert(0, _p)

import numpy as np
import ml_dtypes
from contextlib import ExitStack

# antenv.axon_hooks shim: lets run_bass_kernel_spmd find the NTFF profiling
# hook when BASS_TRACE=1 (the agent image's antenv lacks this module).
import antenv  # noqa: F401

if "antenv.axon_hooks" not in sys.modules:
    _m = types.ModuleType("antenv.axon_hooks")
    _m._hook = None

    def _set_hook(h):
        _m._hook = h

    def _get_hook():
        return _m._hook

    _m.set_axon_ntff_profile_hook = _set_hook
    _m.get_axon_ntff_profile_hook = _get_hook
    sys.modules["antenv.axon_hooks"] = _m
    try:
        from trn_agent_boot.trn_boot import _ntff_profile_via_ctypes

        hook = _ntff_profile_via_ctypes("/opt/axon/libaxon_pjrt.so")
        if hook is not None:
            _set_hook(hook)
    except Exception:
        pass

import concourse.bass as bass  # noqa: E402
import concourse.tile as tile  # noqa: E402
from concourse.tile import add_dep_helper  # noqa: E402
from concourse import mybir, bacc  # noqa: E402
from concourse import bass_utils  # noqa: E402
import concourse.dve_ops as dve_ops_mod  # noqa: E402
from concourse.dve_ops import (  # noqa: E402
    DveOp,
    RECIPROCAL_APPROX_FAST,
    RECIP_APPROX_FAST_CONSTS,
)
from concourse.dve_spec import (  # noqa: E402
    Spec,
    Src0,
    Src1,
    C0,
    C1,
    C2,
    One,
    lower,
    sq,
    _has_src1,
)
from concourse.dve_uop import DveOpSpec  # noqa: E402

# No bucket storage in this container; artifacts stay local.
bass_utils.upload_artifacts = lambda tmpdir: f"local://{tmpdir}"

import os  # noqa: E402

USE_DVE_EXP = os.environ.get("ANT_DVE_EXP", "1") == "1"
USE_FAST_RECIP = os.environ.get("ANT_FAST_RECIP", "1") == "1"
USE_WARMUP = os.environ.get("ANT_WARMUP", "1") == "1"

B, N, C = 2, 4096, 512
H, D = 8, 64
N_CORES = 8
SCALE = D ** -0.5

BF16 = mybir.dt.bfloat16
F32 = mybir.dt.float32
AF = mybir.ActivationFunctionType
BFNP = ml_dtypes.bfloat16

NI = N // 128   # 32 kidx / n chunks
NJ = N // 1024  # 4 q blocks
VW = 2 * (D + 1)  # 130: per-n-chunk vaug block (2 heads x (64 v + 1 ones))

# ---- custom DVE exp: est = exp(s/8) = (q3(s) + d0)^32 ----------------------
# q3 is the odd part of the minimax-relative cubic for e^{s/256} on
# |s| <= 64 (d0 ~= 1 is its constant term); ^32 via 5 squarings. End-to-end
# fp32 rel err 6.5e-4 -- an order of magnitude below the bf16 quantisation
# already in this pipeline. Both ops use only Src0 + scalar slots: on this
# device firmware, custom-DVE programs carrying an `in1` operand or placed
# on opcode rows beyond the production set fail (verified empirically), so
# each op hijacks the opcode row of a production op this kernel never uses
# (the per-NEFF table generator repoints the row at our uOps).
EXP_D0 = 0.9999801457968281
EXP_D1 = 0.0039064241718242045
EXP_D2 = 7.668925879093357e-06
EXP_D3 = 9.903018013391098e-09


def _hijack_dve_op(name, spec, donor):
    for op in dve_ops_mod.OPS:
        if op.name == name:
            return op
    idx = next(i for i, op in enumerate(dve_ops_mod.OPS) if op.name == donor)
    row = dve_ops_mod._CUSTOM_DVE_ROW_BASE + idx
    op = DveOp(name, spec, False, {})
    dve_ops_mod.OPS[idx] = op
    dve_ops_mod._SUB_OPCODE_FOR_NAME[name] = row
    dve_ops_mod.CUSTOM_DVE_SPECS[name] = spec
    for ver in ("v3", "v4"):
        try:
            s = DveOpSpec(
                name=name,
                opcode=row,
                uops=lower(spec, ver=ver),
                rd1_en=_has_src1(spec),
            )
            op.uops_sha[ver] = s.sha(ver)
        except Exception:
            pass
    return op


# q = ((d3*s + d2)*s + d1)*s  -- Horner, 5 ALU stages, scalars only.
EXP_Q3 = _hijack_dve_op(
    "ANT_EXP_Q3",
    Spec(
        body=((C2 * Src0 + C1) * Src0 + C0) * Src0,
        reference=lambda in0, in1, s0, s1, imm2: (
            (imm2 * in0.astype(np.float32) + s1) * in0 + s0
        )
        * in0,
    ),
    "ADD_RANGE_WRAP",
)

# est = (q + d0)^32 -- 6 ALU stages (add + 5 squarings).
_t = Src0 + C0
for _ in range(5):
    _t = sq(_t)
EXP_POW32 = _hijack_dve_op(
    "ANT_P1POW32",
    Spec(
        body=_t,
        reference=lambda in0, in1, s0, s1, imm2: (
            in0.astype(np.float32) + s0
        )
        ** 32,
    ),
    "TENSOR_MASK",
)

_RC = RECIP_APPROX_FAST_CONSTS


def _recip_fast(nc, out_ap, in_ap):
    if not USE_FAST_RECIP:
        with nc.allow_low_precision(reason="softmax 1/den in bf16"):
            return nc.vector.reciprocal(out_ap, in_ap)
    return nc.vector._custom_dve(
        RECIPROCAL_APPROX_FAST,
        out=out_ap,
        in0=in_ap,
        s0=_RC["s0"],
        s1=_RC["s1"],
        imm2=_RC["imm2"],
    )


def _dve_steps(j):
    # Which i-steps of block j compute exp on the Vector engine instead of
    # ScalarE. Block 0 is PE/DVE-heavy (QKV stage A + all 32 v-copies land
    # there), so it keeps exp fully on ScalarE; i==31 stays on ScalarE so the
    # accumulation-stop AV keeps the short pipeline across block boundaries.
    if not USE_DVE_EXP or j == 0:
        return frozenset()
    if j == 1:
        return frozenset((5, 9, 13, 17, 21, 25))
    return frozenset((3, 7, 11, 15, 19, 23, 27))


def build_nc():
    nc = bacc.Bacc("TRN2", target_bir_lowering=False, debug=False)

    xt = nc.dram_tensor("xt", [4, 128, N], BF16, kind="ExternalInput").ap()
    wq = nc.dram_tensor("wq", [128, 512], BF16, kind="ExternalInput").ap()
    wk = nc.dram_tensor("wk", [128, 512], BF16, kind="ExternalInput").ap()
    wv = nc.dram_tensor("wv", [128, 512], BF16, kind="ExternalInput").ap()
    wp = nc.dram_tensor("wp", [64, 1024], BF16, kind="ExternalInput").ap()
    # partials leave the device in bf16: the host sums 4 partials per batch
    # in fp32, and the added quantisation (~1e-3 of absmax) is well inside
    # the error budget, while halving output DMA traffic and PSUM-evacuation
    # cost.
    out = nc.dram_tensor("out", [N, C], BF16, kind="ExternalOutput").ap()

    with tile.TileContext(nc) as tc:
        with ExitStack() as ctx:
            const = ctx.enter_context(tc.tile_pool(name="const", bufs=1))
            sb = ctx.enter_context(tc.tile_pool(name="sb", bufs=1))
            expp = ctx.enter_context(tc.tile_pool(name="expp", bufs=6))
            upool = ctx.enter_context(tc.tile_pool(name="upool", bufs=2))
            invp = ctx.enter_context(tc.tile_pool(name="invp", bufs=2))
            outp = ctx.enter_context(tc.tile_pool(name="outp", bufs=3))

            # memsets on the Vector engine so the warm-up matmuls can start as
            # soon as the engines finish their init preamble.
            tones = const.tile([1, 128], BF16)
            nc.vector.memset(tones[:], 1.0)
            twarm = const.tile([128, 512], BF16)
            nc.vector.memset(twarm[:], 0.5)

            # PE warm-up: dependency-free full-array (K=128) junk matmuls
            # issued while the input DMAs run, so the HAM clock-gate reaches
            # K=8/8 (2.4 GHz) before the first real matmul, instead of ~25us
            # in. K=1 matmuls do not register enough PE activity for HAM.
            if USE_WARMUP:
                with tc.tile_pool(name="warm", bufs=2, space="PSUM") as warmp:
                    for _ in range(12):
                        wps = warmp.tile([128, 512], F32, tag="w", name="warm")
                        nc.tensor.matmul(
                            wps[:], twarm[:, 0:128], twarm[:], start=True,
                            stop=True,
                        )

            # q/k weights ride the front of the Sync hardware DMA queue (the
            # GpSimd queue is a slow software-dynamic ring; Scalar is the
            # second hardware queue and carries part of x).
            twq = const.tile([128, 512], BF16)
            nc.sync.dma_start(twq[:], wq[:])
            twk = const.tile([128, 512], BF16)
            nc.sync.dma_start(twk[:], wk[:])
            twv = const.tile([128, 512], BF16)
            twp = const.tile([64, 1024], BF16)

            qT = sb.tile([128, N], BF16)
            kT = sb.tile([128, N], BF16)
            vaug = sb.tile([128, NI * VW], BF16)
            nc.gpsimd.memset(vaug[:], 1.0)
            outT0 = sb.tile([64, N], BF16)
            outT1 = sb.tile([64, N], BF16)
            outTs = (outT0, outT1)

            xtp = ctx.enter_context(tc.tile_pool(name="xtp", bufs=1))
            psS = ctx.enter_context(tc.tile_pool(name="psS", bufs=2, space="PSUM"))
            psAV = ctx.enter_context(tc.tile_pool(name="psAV", bufs=1, space="PSUM"))
            psT = ctx.enter_context(tc.tile_pool(name="psT", bufs=2, space="PSUM"))

            # ---- stage A: QKV projections ------------------------------
            # Emitted as deadline-scheduled tasks threaded into the first two
            # j-blocks' i-loops (the PE queue is strict FIFO; anything emitted
            # before the first score matmul delays the first exp).
            xts = []
            for k in range(4):
                t = xtp.tile([128, N], BF16, tag=f"xt{k}", name=f"xt{k}")
                xts.append(t)
            # Input DMA split across the two hardware queues (sync + scalar),
            # everything the first qk projections need posted first on both.
            # The dummy exp that preloads the ACT table (~2.7us) is slotted
            # on the Scalar engine right after its column-0 posts; the v/proj
            # weights follow the column-0 x chunks (first needed later).
            # column 0 goes in 512-col halves: the first qk projections need
            # only cols 0:512 of every k-chunk, so they start ~2.5us earlier.
            for half in range(2):
                for k in range(4):
                    cs = bass.ds(half * 512, 512)
                    q = nc.sync if k % 2 == 0 else nc.scalar
                    q.dma_start(xts[k][:, cs], xt[k][:, cs])
                if half == 0:
                    tdume = const.tile([1, 16], BF16)
                    nc.scalar.activation(
                        tdume[:], twarm[0:1, 0:16], AF.Exp, scale=SCALE
                    )
            nc.sync.dma_start(twv[:], wv[:])
            nc.sync.dma_start(twp[:], wp[:])
            for col in range(1, 4):
                for k in range(4):
                    cs = bass.ts(col, N // 4)
                    q = nc.sync if k % 2 == 0 else nc.scalar
                    q.dma_start(xts[k][:, cs], xt[k][:, cs])

            def emit_qk(j8, which):
                s_ = bass.ts(j8, 512)
                w, dst = (twq, qT) if which == "q" else (twk, kT)
                ps = psT.tile([128, 512], F32, tag="t", name="psqk")
                for k in range(4):
                    nc.tensor.matmul(
                        ps[:], w[:, bass.ts(k, 128)], xts[k][:, s_],
                        start=(k == 0), stop=(k == 3),
                    )
                nc.vector.tensor_copy(dst[:, s_], ps[:])

            def emit_v(jj):
                ps = psT.tile([128, 128], F32, tag="t", name="psv")
                for k in range(4):
                    nc.tensor.matmul(
                        ps[:], xts[k][:, bass.ts(jj, 128)], twv[:, bass.ts(k, 128)],
                        start=(k == 0), stop=(k == 3),
                    )
                dst = vaug[:, jj * VW : (jj + 1) * VW].rearrange(
                    "p (h c) -> p h c", h=2
                )[:, :, 0:D]
                src = ps[:].rearrange("p (h c) -> p h c", h=2)
                nc.vector.tensor_copy(dst, src)

            # (deadline in global i-steps, emitter) — qk k-chunk c feeds
            # scores at step 4c; v chunk jj feeds the AV matmul at step jj;
            # qk q-chunk j8 feeds block j8 (step 32*j8).
            stage_a_tasks = []
            for c in range(1, 8):
                stage_a_tasks.append((4 * c - 4, lambda c=c: emit_qk(c, "k")))
            for jj in range(1, NI):
                stage_a_tasks.append((jj - 2, lambda jj=jj: emit_v(jj)))
            for j8 in range(1, 8):
                stage_a_tasks.append((32 * j8 - 10, lambda j8=j8: emit_qk(j8, "q")))
            stage_a_tasks.sort(key=lambda t: t[0])
            stage_a_tasks = list(stage_a_tasks)

            # prologue: what step 0 needs
            emit_qk(0, "q")
            emit_qk(0, "k")
            emit_v(0)

            # ---- stage B: scores^T -> exp -> AV (+den), normalize -------
            # ---- stage C: partial projection --------------------------
            # Tails (normalize + projection of block j) are emitted in the
            # middle of block j+1's i-loop: the PE queue is strict FIFO, so
            # matmuls that wait on slow Vector-engine work must sit behind
            # enough independent PE work to never stall the queue.
            def emit_bcast(st, after=None):
                h = st["h"]
                psb = psT.tile([64, 512], F32, tag="t", name="psb")
                mi = nc.tensor.matmul(
                    psb[:], tones[0:1, 0:64], st["inv"][:], start=True, stop=True
                )
                if after is not None:
                    add_dep_helper(mi.ins, after.ins, sync=False,
                                   reason="tail behind scores")
                sbb = invp.tile([64, 512], BF16, tag="sbb", name="sbb")
                nc.vector.tensor_copy(sbb[:], psb[:])
                nc.vector.tensor_mul(
                    outTs[h][:, st["qs"]], st["avsb"][0:64, :], sbb[:]
                )

            def emit_proj(j, k, after=None, tail=False):
                jj = j * 4 + k
                s = bass.ts(jj, 128)
                pp = psT.tile([128, 512], F32, tag="t", name="pp")
                mi = nc.tensor.matmul(
                    pp[:], outT0[:, s], twp[:, 0:512], start=True, stop=False
                )
                if after is not None:
                    add_dep_helper(mi.ins, after.ins, sync=False,
                                   reason="tail behind scores")
                nc.tensor.matmul(
                    pp[:], outT1[:, s], twp[:, 512:1024], start=False, stop=True
                )
                ot = outp.tile([128, 512], BF16, tag="o", name="ot")
                if tail:
                    # ScalarE is idle once the last exp is done; moving the
                    # evacuation there takes it off the Vector critical path.
                    nc.scalar.copy(ot[:], pp[:])
                else:
                    nc.vector.tensor_copy(ot[:], pp[:])
                q = nc.scalar if (tail and k % 2 == 1) else nc.sync
                q.dma_start(out[s, :], ot[:])

            # Flat software pipeline over all 256 i-steps. AV matmuls are
            # emitted 1 step (ScalarE exp) or 3 steps (Vector exp) behind
            # their scores so the PE FIFO always holds independent score
            # work while an AV's est is still being produced.
            prev = None   # pending normalize/proj tail of the finished block
            pend = []     # AV emissions: dicts with a due slot
            dve_pow = None  # deferred ^32 passes of the previous DVE step
            avs = None
            NT = 8 * NI
            for gs in range(NT + 1):
                j, i = divmod(gs, NI)
                if gs < NT:
                    if i == 0:
                        avs = [
                            psAV.tile([65, 512], F32, tag=f"av{t}", name=f"av{t}")
                            for t in range(2)
                        ]
                    while stage_a_tasks and stage_a_tasks[0][0] <= gs + 3:
                        stage_a_tasks.pop(0)[1]()
                    if prev is not None:
                        if i == 6:
                            emit_bcast(prev["n"][0], after=last_sc)
                        elif i == 10:
                            emit_bcast(prev["n"][1], after=last_sc)
                        elif i >= 16 and i % 4 == 0:  # 16, 20, 24, 28
                            emit_proj(prev["j"], (i - 16) // 4, after=last_sc)
                    qs = bass.ts(j, 512)
                    ks = bass.ts(i, 128)
                    pss = psS.tile([128, 1024], F32, tag="s")
                    nc.tensor.matmul(
                        pss[:, 0:512], kT[0:64, ks], qT[0:64, qs],
                        start=True, stop=True,
                    )
                    last_sc = nc.tensor.matmul(
                        pss[:, 512:1024], kT[64:128, ks], qT[64:128, qs],
                        start=True, stop=True,
                    )
                    # deferred (q+d0)^32 passes from the previous DVE step:
                    # emitting them one slot later keeps the urgent cubic
                    # passes (which release the PSUM score banks) at the
                    # front of the Vector FIFO.
                    if dve_pow is not None:
                        p_u, p_est = dve_pow
                        for h2 in range(2):
                            cs2 = bass.ts(h2, 512)
                            nc.vector._custom_dve(
                                EXP_POW32,
                                out=p_est[:, cs2],
                                in0=p_u[:, cs2],
                                s0=EXP_D0,
                            )
                        dve_pow = None
                    est = expp.tile([128, 1024], BF16, tag="e")
                    if i in _dve_steps(j):
                        # exp on the Vector engine: two [128,512] cubic passes
                        # (frees the PSUM score banks incrementally), then two
                        # (q+d0)^32 passes SBUF->SBUF into bf16 next slot.
                        u = upool.tile([128, 1024], F32, tag="u", name="u")
                        for h2 in range(2):
                            cs2 = bass.ts(h2, 512)
                            nc.vector._custom_dve(
                                EXP_Q3,
                                out=u[:, cs2],
                                in0=pss[:, cs2],
                                s0=EXP_D1,
                                s1=EXP_D2,
                                imm2=EXP_D3,
                            )
                        dve_pow = (u, est)
                        delay = 3
                    else:
                        nc.scalar.activation(est[:], pss[:], AF.Exp, scale=SCALE)
                        # 2 slots, not 1: the ~1.15us ACTIVATE finishes only
                        # marginally before a delay-1 AV's turn in the PE
                        # FIFO, stalling the PE ~0.3-0.4us every step. The
                        # block-final (stop) step keeps delay 1 so the psAV
                        # evacuation starts a slot earlier, unblocking the
                        # next block's first AV.
                        delay = 1 if i == NI - 1 else 2
                    pend.append(
                        {
                            "due": gs + delay,
                            "avs": avs,
                            "est": est,
                            "start": i == 0,
                            "stop": i == NI - 1,
                            "i": i,
                            "qs": qs,
                            "j": j,
                        }
                    )
                while pend and (pend[0]["due"] <= gs or gs == NT):
                    p = pend.pop(0)
                    p_avs, p_est, p_i = p["avs"], p["est"], p["i"]
                    for h in range(2):
                        va = vaug[:, p_i * VW + h * 65 : p_i * VW + (h + 1) * 65]
                        nc.tensor.matmul(
                            p_avs[h][:], va, p_est[:, bass.ts(h, 512)],
                            start=p["start"], stop=p["stop"],
                        )
                    if p["stop"]:
                        # evacuate accumulators fast (releases banks for the
                        # new block) and start the reciprocals; the rest of
                        # the tail goes through the i==6/10/16+ hooks above.
                        # Evacuation copies FIRST (they are all that gates
                        # the psAV bank reuse by the next block's AV), then
                        # the partition-0 den hop + reciprocals off that
                        # critical path.
                        norms = []
                        if gs < NT:
                            for h in range(2):
                                avsb = invp.tile(
                                    [65, 512], F32, tag="avsb", name="avsb"
                                )
                                nc.vector.tensor_copy(avsb[:], p_avs[h][:])
                                norms.append(
                                    {"h": h, "qs": p["qs"], "avsb": avsb}
                                )
                            for st in norms:
                                # den row sits on partition 64; custom-DVE
                                # needs base partition 0, so hop it through
                                # a native copy first.
                                dent = invp.tile(
                                    [1, 512], F32, tag="dent", name="dent"
                                )
                                nc.vector.tensor_copy(
                                    dent[:], st["avsb"][64:65, :]
                                )
                                inv = invp.tile(
                                    [1, 512], BF16, tag="inv", name="inv"
                                )
                                _recip_fast(nc, inv[:], dent[:])
                                st["dent"] = dent
                                st["inv"] = inv
                        else:
                            # last block: skip the evacuation copies (the
                            # tail reads the accumulators in PSUM directly)
                            # and do fine-grained per-128-col den hops and
                            # recips there.
                            for h in range(2):
                                norms.append(
                                    {
                                        "h": h,
                                        "qs": p["qs"],
                                        "avsb": p_avs[h],
                                        "inv": None,
                                    }
                                )
                        prev = {"j": p["j"], "n": norms}
            # final block's tail: fine-grained 128-column pipeline (no next
            # block hides it, so shorten the critical chain instead)
            lj = prev["j"]
            for k in range(4):
                cs = bass.ds(lj * 512 + k * 128, 128)
                for st in prev["n"]:
                    h = st["h"]
                    # per-128-col den hop to partition 0 (custom-DVE needs
                    # base partition 0), then fast reciprocal.
                    dentk = invp.tile([1, 128], F32, tag="dentk", name="dentk")
                    nc.vector.tensor_copy(
                        dentk[:], st["avsb"][64:65, bass.ts(k, 128)]
                    )
                    invk = invp.tile([1, 128], BF16, tag="invk", name="invk")
                    _recip_fast(nc, invk[:], dentk[:])
                    # bcast PSUM rides the (now idle) score pool so it
                    # double-buffers independently of the proj pool.
                    psbt = psS.tile([128, 1024], F32, tag="s", name="psbt")
                    psb = psbt[0:64, 0:128]
                    nc.tensor.matmul(
                        psb, tones[0:1, 0:64], invk[:], start=True, stop=True
                    )
                    sbb = invp.tile([64, 128], BF16, tag="sbbk", name="sbbk")
                    nc.scalar.copy(sbb[:], psb)
                    nc.vector.tensor_mul(
                        outTs[h][:, cs], st["avsb"][0:64, bass.ts(k, 128)], sbb[:]
                    )
                emit_proj(lj, k, tail=True)

    nc.compile()
    return nc


def _pack_w(wslice):
    # [512, 128] -> SBUF image [128, 4*128] with C-chunk k at cols k*128..
    return np.ascontiguousarray(
        wslice.reshape(4, 128, 128).transpose(1, 0, 2).reshape(128, 512)
    ).astype(BFNP)


_NC_CACHE = None
LAST_RESULT = None


def kernel(x, W_qkv, W_proj, b_proj):
    global _NC_CACHE, LAST_RESULT
    x = np.asarray(x, dtype=np.float32)
    W_qkv = np.asarray(W_qkv, dtype=np.float32)
    W_proj = np.asarray(W_proj, dtype=np.float32)
    b_proj = np.asarray(b_proj, dtype=np.float32)

    if _NC_CACHE is None:
        _NC_CACHE = build_nc()
    nc = _NC_CACHE

    in_maps = []
    for c in range(N_CORES):
        b = c // 4
        h0 = 2 * (c % 4)
        xtb = np.ascontiguousarray(x[b].T).reshape(4, 128, N).astype(BFNP)
        wq = _pack_w(W_qkv[:, h0 * 64 : h0 * 64 + 128])
        wk = _pack_w(W_qkv[:, 512 + h0 * 64 : 512 + h0 * 64 + 128])
        wv = _pack_w(W_qkv[:, 1024 + h0 * 64 : 1024 + h0 * 64 + 128])
        wp = np.ascontiguousarray(
            np.concatenate(
                [
                    W_proj[h0 * 64 : (h0 + 1) * 64, :],
                    W_proj[(h0 + 1) * 64 : (h0 + 2) * 64, :],
                ],
                axis=1,
            )
        ).astype(BFNP)
        in_maps.append({"xt": xtb, "wq": wq, "wk": wk, "wv": wv, "wp": wp})

    res = bass_utils.run_bass_kernel_spmd(
        nc, in_maps, core_ids=list(range(N_CORES))
    )
    LAST_RESULT = res

    out = np.zeros((B, N, C), dtype=np.float32)
    for c in range(N_CORES):
        out[c // 4] += res.results[c]["out"].astype(np.float32)
    out += b_proj[None, None, :]
    return out


# revision 58
# speedup vs baseline: 1.0039x; 1.0039x over previous
"""Multi-head attention (B=2, N=4096, C=512, H=8, d=64) on 8 Trainium2 NeuronCores.

Sharding: core c handles batch b = c//4 and heads {2*(c%4), 2*(c%4)+1}.
Each core computes its 2 heads' attention plus a partial output projection
(contraction over its 128 rows of W_proj); the host gather sums the 4
partials per batch and adds the bias.

On-device dataflow (transposed-scores formulation, no on-chip transposes):
  qT/kT [128=2*64 d-dims, 4096]  = W.T @ x.T      (x.T supplied by host)
  v_aug [128 n-chunk, 32*(65+65)] = x @ Wv with a ones column per head
  S^T[kidx, q] = kT.T_chunk @ qT  (two heads ride row-groups 0-1 / 2-3
                                   of the PE array concurrently, K=64 each)
  E = exp(S^T / 8)                softmax exp split across TWO engines:
                                  most steps on ScalarE (LUT exp, scale
                                  folded), ~7/32 steps per block on the
                                  Vector engine via two custom DVE ops:
                                  q = cubic(s) (minimax for e^{s/256})
                                  then est = (q + d0)^32 (5 squarings).
                                  ScalarE is the critical engine
                                  (1 elem/lane/cycle); offloading part
                                  of the exp volume to DVE breaks that
                                  ceiling.
  [out_unnorm^T; den] = v_aug.T @ E   (ones column makes row 64 the softmax
                                       denominator -- no extra pass)
  out^T = out_unnorm^T * (1/den)  (fast approx reciprocal + K=1 broadcast
                                   matmul)
  partial = out^T.T @ W_proj_slice  (per-head K=64 contractions; bias is
                                     added by the host gather)
"""

import sys
import types

for _p in ("/opt/trn_rl_repo",):
    if _p not in sys.path:
        sys.path.insert(0, _p)

import numpy as np
import ml_dtypes
from contextlib import ExitStack

# antenv.axon_hooks shim: lets run_bass_kernel_spmd find the NTFF profiling
# hook when BASS_TRACE=1 (the agent image's antenv lacks this module).
import antenv  # noqa: F401

if "antenv.axon_hooks" not in sys.modules:
    _m = types.ModuleType("antenv.axon_hooks")
    _m._hook = None

    def _set_hook(h):
        _m._hook = h

    def _get_hook():
        return _m._hook

    _m.set_axon_ntff_profile_hook = _set_hook
    _m.get_axon_ntff_profile_hook = _get_hook
    sys.modules["antenv.axon_hooks"] = _m
    try:
        from trn_agent_boot.trn_boot import _ntff_profile_via_ctypes

        hook = _ntff_profile_via_ctypes("/opt/axon/libaxon_pjrt.so")
        if hook is not None:
            _set_hook(hook)
    except Exception:
        pass

import concourse.bass as bass  # noqa: E402
import concourse.tile as tile  # noqa: E402
from concourse.tile import add_dep_helper  # noqa: E402
from concourse import mybir, bacc  # noqa: E402
from concourse import bass_utils  # noqa: E402
import concourse.dve_ops as dve_ops_mod  # noqa: E402
from concourse.dve_ops import (  # noqa: E402
    DveOp,
    RECIPROCAL_APPROX_FAST,
    RECIP_APPROX_FAST_CONSTS,
)
from concourse.dve_spec import (  # noqa: E402
    Spec,
    Src0,
    Src1,
    C0,
    C1,
    C2,
    One,
    lower,
    sq,
    _has_src1,
)
from concourse.dve_uop import DveOpSpec  # noqa: E402

# No bucket storage in this container; artifacts stay local.
bass_utils.upload_artifacts = lambda tmpdir: f"local://{tmpdir}"

import os  # noqa: E402

USE_DVE_EXP = os.environ.get("ANT_DVE_EXP", "1") == "1"
USE_FAST_RECIP = os.environ.get("ANT_FAST_RECIP", "1") == "1"
USE_WARMUP = os.environ.get("ANT_WARMUP", "1") == "1"

B, N, C = 2, 4096, 512
H, D = 8, 64
N_CORES = 8
SCALE = D ** -0.5

BF16 = mybir.dt.bfloat16
F32 = mybir.dt.float32
AF = mybir.ActivationFunctionType
BFNP = ml_dtypes.bfloat16

NI = N // 128   # 32 kidx / n chunks
NJ = N // 1024  # 4 q blocks
VW = 2 * (D + 1)  # 130: per-n-chunk vaug block (2 heads x (64 v + 1 ones))

# ---- custom DVE exp: est = exp(s/8) = (q3(s) + d0)^32 ----------------------
# q3 is the odd part of the minimax-relative cubic for e^{s/256} on
# |s| <= 64 (d0 ~= 1 is its constant term); ^32 via 5 squarings. End-to-end
# fp32 rel err 6.5e-4 -- an order of magnitude below the bf16 quantisation
# already in this pipeline. Both ops use only Src0 + scalar slots: on this
# device firmware, custom-DVE programs carrying an `in1` operand or placed
# on opcode rows beyond the production set fail (verified empirically), so
# each op hijacks the opcode row of a production op this kernel never uses
# (the per-NEFF table generator repoints the row at our uOps).
EXP_D0 = 0.9999801457968281
EXP_D1 = 0.0039064241718242045
EXP_D2 = 7.668925879093357e-06
EXP_D3 = 9.903018013391098e-09


def _hijack_dve_op(name, spec, donor):
    for op in dve_ops_mod.OPS:
        if op.name == name:
            return op
    idx = next(i for i, op in enumerate(dve_ops_mod.OPS) if op.name == donor)
    row = dve_ops_mod._CUSTOM_DVE_ROW_BASE + idx
    op = DveOp(name, spec, False, {})
    dve_ops_mod.OPS[idx] = op
    dve_ops_mod._SUB_OPCODE_FOR_NAME[name] = row
    dve_ops_mod.CUSTOM_DVE_SPECS[name] = spec
    for ver in ("v3", "v4"):
        try:
            s = DveOpSpec(
                name=name,
                opcode=row,
                uops=lower(spec, ver=ver),
                rd1_en=_has_src1(spec),
            )
            op.uops_sha[ver] = s.sha(ver)
        except Exception:
            pass
    return op


# q = ((d3*s + d2)*s + d1)*s  -- Horner, 5 ALU stages, scalars only.
EXP_Q3 = _hijack_dve_op(
    "ANT_EXP_Q3",
    Spec(
        body=((C2 * Src0 + C1) * Src0 + C0) * Src0,
        reference=lambda in0, in1, s0, s1, imm2: (
            (imm2 * in0.astype(np.float32) + s1) * in0 + s0
        )
        * in0,
    ),
    "ADD_RANGE_WRAP",
)

# est = (q + d0)^32 -- 6 ALU stages (add + 5 squarings).
_t = Src0 + C0
for _ in range(5):
    _t = sq(_t)
EXP_POW32 = _hijack_dve_op(
    "ANT_P1POW32",
    Spec(
        body=_t,
        reference=lambda in0, in1, s0, s1, imm2: (
            in0.astype(np.float32) + s0
        )
        ** 32,
    ),
    "TENSOR_MASK",
)

_RC = RECIP_APPROX_FAST_CONSTS


def _recip_fast(nc, out_ap, in_ap):
    if not USE_FAST_RECIP:
        with nc.allow_low_precision(reason="softmax 1/den in bf16"):
            return nc.vector.reciprocal(out_ap, in_ap)
    return nc.vector._custom_dve(
        RECIPROCAL_APPROX_FAST,
        out=out_ap,
        in0=in_ap,
        s0=_RC["s0"],
        s1=_RC["s1"],
        imm2=_RC["imm2"],
    )


def _dve_steps(j):
    # Which i-steps of block j compute exp on the Vector engine instead of
    # ScalarE. Block 0 is PE/DVE-heavy (QKV stage A + all 32 v-copies land
    # there), so it keeps exp fully on ScalarE; i==31 stays on ScalarE so the
    # accumulation-stop AV keeps the short pipeline across block boundaries.
    if not USE_DVE_EXP or j == 0:
        return frozenset()
    if j == 1:
        return frozenset((7, 12, 17, 22, 27))
    return frozenset((3, 7, 11, 15, 19, 23, 27))


def build_nc():
    nc = bacc.Bacc("TRN2", target_bir_lowering=False, debug=False)

    xt = nc.dram_tensor("xt", [4, 128, N], BF16, kind="ExternalInput").ap()
    wq = nc.dram_tensor("wq", [128, 512], BF16, kind="ExternalInput").ap()
    wk = nc.dram_tensor("wk", [128, 512], BF16, kind="ExternalInput").ap()
    wv = nc.dram_tensor("wv", [128, 512], BF16, kind="ExternalInput").ap()
    wp = nc.dram_tensor("wp", [64, 1024], BF16, kind="ExternalInput").ap()
    # partials leave the device in bf16: the host sums 4 partials per batch
    # in fp32, and the added quantisation (~1e-3 of absmax) is well inside
    # the error budget, while halving output DMA traffic and PSUM-evacuation
    # cost.
    out = nc.dram_tensor("out", [N, C], BF16, kind="ExternalOutput").ap()

    with tile.TileContext(nc) as tc:
        with ExitStack() as ctx:
            const = ctx.enter_context(tc.tile_pool(name="const", bufs=1))
            sb = ctx.enter_context(tc.tile_pool(name="sb", bufs=1))
            expp = ctx.enter_context(tc.tile_pool(name="expp", bufs=8))
            upool = ctx.enter_context(tc.tile_pool(name="upool", bufs=3))
            invp = ctx.enter_context(tc.tile_pool(name="invp", bufs=2))
            outp = ctx.enter_context(tc.tile_pool(name="outp", bufs=3))

            # memsets on the Vector engine so the warm-up matmuls can start as
            # soon as the engines finish their init preamble.
            tones = const.tile([1, 128], BF16)
            nc.vector.memset(tones[:], 1.0)
            twarm = const.tile([128, 512], BF16)
            nc.vector.memset(twarm[:], 0.5)

            # PE warm-up: dependency-free full-array (K=128) junk matmuls
            # issued while the input DMAs run, so the HAM clock-gate reaches
            # K=8/8 (2.4 GHz) before the first real matmul, instead of ~25us
            # in. K=1 matmuls do not register enough PE activity for HAM.
            if USE_WARMUP:
                with tc.tile_pool(name="warm", bufs=2, space="PSUM") as warmp:
                    for _ in range(9):
                        wps = warmp.tile([128, 512], F32, tag="w", name="warm")
                        nc.tensor.matmul(
                            wps[:], twarm[:, 0:128], twarm[:], start=True,
                            stop=True,
                        )

            # q/k weights ride the front of the Sync hardware DMA queue (the
            # GpSimd queue is a slow software-dynamic ring; Scalar is the
            # second hardware queue and carries part of x).
            twq = const.tile([128, 512], BF16)
            nc.sync.dma_start(twq[:], wq[:])
            twk = const.tile([128, 512], BF16)
            nc.sync.dma_start(twk[:], wk[:])
            twv = const.tile([128, 512], BF16)
            twp = const.tile([64, 1024], BF16)

            qT = sb.tile([128, N], BF16)
            kT = sb.tile([128, N], BF16)
            vaug = sb.tile([128, NI * VW], BF16)
            nc.gpsimd.memset(vaug[:], 1.0)
            outT0 = sb.tile([64, N], BF16)
            outT1 = sb.tile([64, N], BF16)
            outTs = (outT0, outT1)

            xtp = ctx.enter_context(tc.tile_pool(name="xtp", bufs=1))
            psS = ctx.enter_context(tc.tile_pool(name="psS", bufs=2, space="PSUM"))
            psAV = ctx.enter_context(tc.tile_pool(name="psAV", bufs=1, space="PSUM"))
            psT = ctx.enter_context(tc.tile_pool(name="psT", bufs=2, space="PSUM"))

            # ---- stage A: QKV projections ------------------------------
            # Emitted as deadline-scheduled tasks threaded into the first two
            # j-blocks' i-loops (the PE queue is strict FIFO; anything emitted
            # before the first score matmul delays the first exp).
            xts = []
            for k in range(4):
                t = xtp.tile([128, N], BF16, tag=f"xt{k}", name=f"xt{k}")
                xts.append(t)
            # Input DMA split across the two hardware queues (sync + scalar),
            # everything the first qk projections need posted first on both.
            # The dummy exp that preloads the ACT table (~2.7us) is slotted
            # on the Scalar engine right after its column-0 posts; the v/proj
            # weights follow the column-0 x chunks (first needed later).
            # column 0 goes in 512-col halves: the first qk projections need
            # only cols 0:512 of every k-chunk, so they start ~2.5us earlier.
            for half in range(2):
                for k in range(4):
                    cs = bass.ds(half * 512, 512)
                    q = nc.sync if k % 2 == 0 else nc.scalar
                    q.dma_start(xts[k][:, cs], xt[k][:, cs])
                if half == 0:
                    tdume = const.tile([1, 16], BF16)
                    nc.scalar.activation(
                        tdume[:], twarm[0:1, 0:16], AF.Exp, scale=SCALE
                    )
            nc.sync.dma_start(twv[:], wv[:])
            nc.sync.dma_start(twp[:], wp[:])
            for col in range(1, 4):
                for k in range(4):
                    cs = bass.ts(col, N // 4)
                    q = nc.sync if k % 2 == 0 else nc.scalar
                    q.dma_start(xts[k][:, cs], xt[k][:, cs])

            def emit_qk(j8, which):
                s_ = bass.ts(j8, 512)
                w, dst = (twq, qT) if which == "q" else (twk, kT)
                ps = psT.tile([128, 512], F32, tag="t", name="psqk")
                for k in range(4):
                    nc.tensor.matmul(
                        ps[:], w[:, bass.ts(k, 128)], xts[k][:, s_],
                        start=(k == 0), stop=(k == 3),
                    )
                nc.vector.tensor_copy(dst[:, s_], ps[:])

            def emit_v(jj):
                ps = psT.tile([128, 128], F32, tag="t", name="psv")
                for k in range(4):
                    nc.tensor.matmul(
                        ps[:], xts[k][:, bass.ts(jj, 128)], twv[:, bass.ts(k, 128)],
                        start=(k == 0), stop=(k == 3),
                    )
                dst = vaug[:, jj * VW : (jj + 1) * VW].rearrange(
                    "p (h c) -> p h c", h=2
                )[:, :, 0:D]
                src = ps[:].rearrange("p (h c) -> p h c", h=2)
                nc.vector.tensor_copy(dst, src)

            # (deadline in global i-steps, emitter) — qk k-chunk c feeds
            # scores at step 4c; v chunk jj feeds the AV matmul at step jj;
            # qk q-chunk j8 feeds block j8 (step 32*j8).
            stage_a_tasks = []
            for c in range(1, 8):
                stage_a_tasks.append((4 * c - 4, lambda c=c: emit_qk(c, "k")))
            for jj in range(1, NI):
                stage_a_tasks.append((jj - 2, lambda jj=jj: emit_v(jj)))
            for j8 in range(1, 8):
                stage_a_tasks.append((32 * j8 - 10, lambda j8=j8: emit_qk(j8, "q")))
            stage_a_tasks.sort(key=lambda t: t[0])
            stage_a_tasks = list(stage_a_tasks)

            # prologue: what step 0 needs
            emit_qk(0, "q")
            emit_qk(0, "k")
            emit_v(0)

            # ---- stage B: scores^T -> exp -> AV (+den), normalize -------
            # ---- stage C: partial projection --------------------------
            # Tails (normalize + projection of block j) are emitted in the
            # middle of block j+1's i-loop: the PE queue is strict FIFO, so
            # matmuls that wait on slow Vector-engine work must sit behind
            # enough independent PE work to never stall the queue.
            def emit_bcast(st, after=None):
                h = st["h"]
                psb = psT.tile([64, 512], F32, tag="t", name="psb")
                mi = nc.tensor.matmul(
                    psb[:], tones[0:1, 0:64], st["inv"][:], start=True, stop=True
                )
                if after is not None:
                    add_dep_helper(mi.ins, after.ins, sync=False,
                                   reason="tail behind scores")
                sbb = invp.tile([64, 512], BF16, tag="sbb", name="sbb")
                nc.vector.tensor_copy(sbb[:], psb[:])
                nc.vector.tensor_mul(
                    outTs[h][:, st["qs"]], st["avsb"][0:64, :], sbb[:]
                )

            def emit_proj(j, k, after=None, tail=False):
                jj = j * 4 + k
                s = bass.ts(jj, 128)
                pp = psT.tile([128, 512], F32, tag="t", name="pp")
                mi = nc.tensor.matmul(
                    pp[:], outT0[:, s], twp[:, 0:512], start=True, stop=False
                )
                if after is not None:
                    add_dep_helper(mi.ins, after.ins, sync=False,
                                   reason="tail behind scores")
                nc.tensor.matmul(
                    pp[:], outT1[:, s], twp[:, 512:1024], start=False, stop=True
                )
                ot = outp.tile([128, 512], BF16, tag="o", name="ot")
                if tail:
                    # ScalarE is idle once the last exp is done; moving the
                    # evacuation there takes it off the Vector critical path.
                    nc.scalar.copy(ot[:], pp[:])
                else:
                    nc.vector.tensor_copy(ot[:], pp[:])
                q = nc.scalar if (tail and k % 2 == 1) else nc.sync
                q.dma_start(out[s, :], ot[:])

            # Flat software pipeline over all 256 i-steps. AV matmuls are
            # emitted 1 step (ScalarE exp) or 3 steps (Vector exp) behind
            # their scores so the PE FIFO always holds independent score
            # work while an AV's est is still being produced.
            prev = None   # pending normalize/proj tail of the finished block
            pend = []     # AV emissions: dicts with a due slot
            dve_pow = None  # deferred ^32 passes of the previous DVE step
            avs = None
            NT = 8 * NI
            for gs in range(NT + 1):
                j, i = divmod(gs, NI)
                if gs < NT:
                    if i == 0:
                        avs = [
                            psAV.tile([65, 512], F32, tag=f"av{t}", name=f"av{t}")
                            for t in range(2)
                        ]
                    while stage_a_tasks and stage_a_tasks[0][0] <= gs + 3:
                        stage_a_tasks.pop(0)[1]()
                    if prev is not None:
                        if i == 6:
                            emit_bcast(prev["n"][0], after=last_sc)
                        elif i == 10:
                            emit_bcast(prev["n"][1], after=last_sc)
                        elif i >= 16 and i % 4 == 0:  # 16, 20, 24, 28
                            emit_proj(prev["j"], (i - 16) // 4, after=last_sc)
                    qs = bass.ts(j, 512)
                    ks = bass.ts(i, 128)
                    pss = psS.tile([128, 1024], F32, tag="s")
                    nc.tensor.matmul(
                        pss[:, 0:512], kT[0:64, ks], qT[0:64, qs],
                        start=True, stop=True,
                    )
                    last_sc = nc.tensor.matmul(
                        pss[:, 512:1024], kT[64:128, ks], qT[64:128, qs],
                        start=True, stop=True,
                    )
                    # deferred (q+d0)^32 passes from the previous DVE step:
                    # emitting them one slot later keeps the urgent cubic
                    # passes (which release the PSUM score banks) at the
                    # front of the Vector FIFO.
                    if dve_pow is not None:
                        p_u, p_est = dve_pow
                        for h2 in range(2):
                            cs2 = bass.ts(h2, 512)
                            nc.vector._custom_dve(
                                EXP_POW32,
                                out=p_est[:, cs2],
                                in0=p_u[:, cs2],
                                s0=EXP_D0,
                            )
                        dve_pow = None
                    est = expp.tile([128, 1024], BF16, tag="e")
                    if i in _dve_steps(j):
                        # exp on the Vector engine: two [128,512] cubic passes
                        # (frees the PSUM score banks incrementally), then two
                        # (q+d0)^32 passes SBUF->SBUF into bf16 next slot.
                        u = upool.tile([128, 1024], F32, tag="u", name="u")
                        for h2 in range(2):
                            cs2 = bass.ts(h2, 512)
                            nc.vector._custom_dve(
                                EXP_Q3,
                                out=u[:, cs2],
                                in0=pss[:, cs2],
                                s0=EXP_D1,
                                s1=EXP_D2,
                                imm2=EXP_D3,
                            )
                        dve_pow = (u, est)
                        delay = 3
                    else:
                        nc.scalar.activation(est[:], pss[:], AF.Exp, scale=SCALE)
                        # 2 slots, not 1: the ~1.15us ACTIVATE finishes only
                        # marginally before a delay-1 AV's turn in the PE
                        # FIFO, stalling the PE ~0.3-0.4us every step. The
                        # block-final (stop) step keeps delay 1 so the psAV
                        # evacuation starts a slot earlier, unblocking the
                        # next block's first AV.
                        delay = 1 if i == NI - 1 else 2
                    pend.append(
                        {
                            "due": gs + delay,
                            "avs": avs,
                            "est": est,
                            "start": i == 0,
                            "stop": i == NI - 1,
                            "i": i,
                            "qs": qs,
                            "j": j,
                        }
                    )
                while pend and (pend[0]["due"] <= gs or gs == NT):
                    p = pend.pop(0)
                    p_avs, p_est, p_i = p["avs"], p["est"], p["i"]
                    for h in range(2):
                        va = vaug[:, p_i * VW + h * 65 : p_i * VW + (h + 1) * 65]
                        nc.tensor.matmul(
                            p_avs[h][:], va, p_est[:, bass.ts(h, 512)],
                            start=p["start"], stop=p["stop"],
                        )
                    if p["stop"]:
                        # evacuate accumulators fast (releases banks for the
                        # new block) and start the reciprocals; the rest of
                        # the tail goes through the i==6/10/16+ hooks above.
                        # Evacuation copies FIRST (they are all that gates
                        # the psAV bank reuse by the next block's AV), then
                        # the partition-0 den hop + reciprocals off that
                        # critical path.
                        norms = []
                        if gs < NT:
                            for h in range(2):
                                avsb = invp.tile(
                                    [65, 512], F32, tag="avsb", name="avsb"
                                )
                                nc.vector.tensor_copy(avsb[:], p_avs[h][:])
                                norms.append(
                                    {"h": h, "qs": p["qs"], "avsb": avsb}
                                )
                            for st in norms:
                                # den row sits on partition 64; custom-DVE
                                # needs base partition 0, so hop it through
                                # a native copy first.
                                dent = invp.tile(
                                    [1, 512], F32, tag="dent", name="dent"
                                )
                                nc.vector.tensor_copy(
                                    dent[:], st["avsb"][64:65, :]
                                )
                                inv = invp.tile(
                                    [1, 512], BF16, tag="inv", name="inv"
                                )
                                _recip_fast(nc, inv[:], dent[:])
                                st["dent"] = dent
                                st["inv"] = inv
                        else:
                            # last block: skip the evacuation copies (the
                            # tail reads the accumulators in PSUM directly)
                            # and do fine-grained per-128-col den hops and
                            # recips there.
                            for h in range(2):
                                norms.append(
                                    {
                                        "h": h,
                                        "qs": p["qs"],
                                        "avsb": p_avs[h],
                                        "inv": None,
                                    }
                                )
                        prev = {"j": p["j"], "n": norms}
            # final block's tail: fine-grained 128-column pipeline (no next
            # block hides it, so shorten the critical chain instead)
            lj = prev["j"]
            for k in range(4):
                cs = bass.ds(lj * 512 + k * 128, 128)
                for st in prev["n"]:
                    h = st["h"]
                    # per-128-col den hop to partition 0 (custom-DVE needs
                    # base partition 0), then fast reciprocal.
                    dentk = invp.tile([1, 128], F32, tag="dentk", name="dentk")
                    nc.vector.tensor_copy(
                        dentk[:], st["avsb"][64:65, bass.ts(k, 128)]
                    )
                    invk = invp.tile([1, 128], BF16, tag="invk", name="invk")
                    _recip_fast(nc, invk[:], dentk[:])
                    # bcast PSUM rides the (now idle) score pool so it
                    # double-buffers independently of the proj pool.
                    psbt = psS.tile([128, 1024], F32, tag="s", name="psbt")
                    psb = psbt[0:64, 0:128]
                    nc.tensor.matmul(
                        psb, tones[0:1, 0:64], invk[:], start=True, stop=True
                    )
                    sbb = invp.tile([64, 128], BF16, tag="sbbk", name="sbbk")
                    nc.scalar.copy(sbb[:], psb)
                    nc.vector.tensor_mul(
                        outTs[h][:, cs], st["avsb"][0:64, bass.ts(k, 128)], sbb[:]
                    )
                emit_proj(lj, k, tail=True)

    nc.compile()
    return nc


def _pack_w(wslice):
    # [512, 128] -> SBUF image [128, 4*128] with C-chunk k at cols k*128..
    return np.ascontiguousarray(
        wslice.reshape(4, 128, 128).transpose(1, 0, 2).reshape(128, 512)
    ).astype(BFNP)


_NC_CACHE = None
LAST_RESULT = None


def kernel(x, W_qkv, W_proj, b_proj):
    global _NC_CACHE, LAST_RESULT
    x = np.asarray(x, dtype=np.float32)
    W_qkv = np.asarray(W_qkv, dtype=np.float32)
    W_proj = np.asarray(W_proj, dtype=np.float32)
    b_proj = np.asarray(b_proj, dtype=np.float32)

    if _NC_CACHE is None:
        _NC_CACHE = build_nc()
    nc = _NC_CACHE

    in_maps = []
    for c in range(N_CORES):
        b = c // 4
        h0 = 2 * (c % 4)
        xtb = np.ascontiguousarray(x[b].T).reshape(4, 128, N).astype(BFNP)
        wq = _pack_w(W_qkv[:, h0 * 64 : h0 * 64 + 128])
        wk = _pack_w(W_qkv[:, 512 + h0 * 64 : 512 + h0 * 64 + 128])
        wv = _pack_w(W_qkv[:, 1024 + h0 * 64 : 1024 + h0 * 64 + 128])
        wp = np.ascontiguousarray(
            np.concatenate(
                [
                    W_proj[h0 * 64 : (h0 + 1) * 64, :],
                    W_proj[(h0 + 1) * 64 : (h0 + 2) * 64, :],
                ],
                axis=1,
            )
        ).astype(BFNP)
        in_maps.append({"xt": xtb, "wq": wq, "wk": wk, "wv": wv, "wp": wp})

    res = bass_utils.run_bass_kernel_spmd(
        nc, in_maps, core_ids=list(range(N_CORES))
    )
    LAST_RESULT = res

    out = np.zeros((B, N, C), dtype=np.float32)
    for c in range(N_CORES):
        out[c // 4] += res.results[c]["out"].astype(np.float32)
    out += b_proj[None, None, :]
    return out
